# revision 3
# baseline (speedup 1.0000x reference)
"""Distributed Trainium2 (Bass/Tile) kernel for the contrastive loss, v2.

Strategy (8 NeuronCores, SPMD, row-sharded similarity matrix):
  Core c owns 512 of the 4096 rows of sim = reps @ reps^T (per l).
  The host rolls the column order by c*512 for each core so a single
  NEFF serves all cores, and pre-casts the embeddings to bf16.
  Each core, per l:
    - DMAs all 4096 raw bf16 embedding rows in [row, d] layout,
    - sums-of-squares per row (fused square+row-sum stt, DVE/Pool split),
    - inv-norm via exp(-0.5*ln(ssq)) on ACT (one table set),
    - normalizes rows (tensor_scalar per 128-row chunk, DVE/Pool split),
    - DMA-TRANSPOSES the normalized rows to [d, row] layout (xbar DMA
      transpose, no PE/PSUM involvement),
    - computes its 512x4096 sim row-block as bf16 matmuls (1 cyc/row)
      into two ping-pong [128, 2048] PSUM tiles (4 banks each),
    - exp(sim/T) on ACT with fused row-sum accum (denominator partials),
    - extracts the self/positive diagonals from the exp-domain SBUF
      output via masked stt against identity (DVE),
  then a small tail combines denom = sum - selfexp, ln's on ACT, and
  weights by joint_valid; host sums the 8 partial tensors (all-reduce).

  The prep work (ssq/invn/normalize/transpose) for layer l+1 is emitted
  one step ahead of simexp(l) so it runs on otherwise-idle DVE/Pool/DMA
  capacity underneath the ACT-bound exp phase; l=0's prep is sliced into
  quarters to shorten the serial lead-in chain.
"""

import numpy as np

TEMP = 0.2
L, B, K, D = 4, 64, 32, 128
N = B * K          # 2048
M = 2 * N          # 4096 rows of sim per l
NCORES = 8
R = M // NCORES    # 512 local rows per core
SEG = M // 128     # 32 row-tiles of 128 per l
INV_T = 1.0 / TEMP

_built = None


def _build():
    global _built
    if _built is not None:
        return _built
    from contextlib import ExitStack

    import concourse.tile as tile
    from concourse import bacc
    import concourse.mybir as mybir
    from concourse.masks import make_identity

    f32 = mybir.dt.float32
    bf16 = mybir.dt.bfloat16
    AF = mybir.ActivationFunctionType
    OP = mybir.AluOpType
    AX = mybir.AxisListType

    # Pin every ACT op to the natural_log_exp_and_others table set (covers
    # Copy/Exp/Identity/Ln/Square), so bacc emits exactly one LoadActFuncSet.
    from concourse import hw_specs as _hw
    _tabs = dict(_hw.get_activation_tables("gen3"))
    _pinned = {
        name: (fns if name == "natural_log_exp_and_others" else frozenset())
        for name, fns in _tabs.items()
    }
    _hw.get_activation_tables.cache_clear()
    _orig = _hw.get_activation_tables.__wrapped__

    def _patched(arch):
        if arch == "gen3":
            return _pinned
        return _orig(arch)

    _hw.get_activation_tables = _patched
    import concourse.bacc as _baccmod
    if hasattr(_baccmod, "get_activation_tables"):
        _baccmod.get_activation_tables = _patched

    nc = bacc.Bacc(None, target_bir_lowering=False)
    emb = nc.dram_tensor("emb_nat", [128, L, SEG, D], bf16, kind="ExternalInput")
    jvl = nc.dram_tensor("jv_local", [R], f32, kind="ExternalInput")
    out = nc.dram_tensor("out_wlp", [128, 4 * L], f32, kind="ExternalOutput")

    with ExitStack() as ctx:
        tc = ctx.enter_context(tile.TileContext(nc))
        singles = ctx.enter_context(tc.tile_pool(name="singles", bufs=1))
        junkp = ctx.enter_context(tc.tile_pool(name="junk", bufs=6))
        sqp = ctx.enter_context(tc.tile_pool(name="sqp", bufs=2))
        expp = ctx.enter_context(tc.tile_pool(name="expo", bufs=12))
        simp = ctx.enter_context(tc.tile_pool(name="sim", bufs=2, space="PSUM"))
        sqs = {}

        ident = singles.tile([128, 128], bf16)
        make_identity(nc, ident[:])

        # dummy ACT op on an immediately-ready tile: pulls the (single)
        # LoadActFuncSet to t~0 instead of mid-way into the l=0 prep chain
        actwarm = singles.tile([128, 128], f32)
        nc.scalar.activation(out=actwarm[:], in_=ident[:], func=AF.Exp)

        w = singles.tile([128, 4], f32)

        natall = singles.tile([128, L, SEG, D], bf16)    # raw rows
        natnall = singles.tile([128, L, SEG, D], bf16)   # normalized rows
        # normalized reps, transposed to [d, (l, s, row)] layout
        xall = singles.tile([128, L, SEG, 128], bf16)
        ssqa = singles.tile([128, L * SEG], f32)
        lnsa = singles.tile([128, L * SEG], f32)
        invna = singles.tile([128, L * SEG], f32)
        # exp row-sum partials: up to 4 tile-pieces per (l, rb)
        dacc = singles.tile([128, 4 * L, 4], f32)
        nc.gpsimd.memset(dacc[:], 0.0)
        sexpb = singles.tile([128, 4 * L], f32)     # exp(self/T)
        pexpb = singles.tile([128, 4 * L], f32)     # exp(pos/T)

        # l=0 arrives in quarter/half slices on the SP HWDGE queue so its
        # first ssq ops can start ~2us in. The later nat DMAs are emitted
        # interleaved with the l0/l1 transposes (SP queue order == issue
        # order) so they don't hog the shared DMA engines during lead-in.
        for a, b in ((0, 8), (8, 16), (16, 32)):
            nc.sync.dma_start(out=natall[:, 0, a:b], in_=emb[:, 0, a:b])

        def natdma(l, gate_l, gate_s):
            """Input DMA for layer l, gated so it cannot transfer before
            chunk (gate_l, gate_s) of the normalize stage is written. The
            dummy copy creates a WAW dependency into the DMA's own output
            region (the DMA then overwrites it), which is the only gating
            the greedy tile scheduler respects — queue order is not FIFO.
            Without this, these dependency-free DMAs hog the shared DMA
            engines ahead of the critical l0 transposes."""
            nc.gpsimd.tensor_copy(natall[:, l, 0], natnall[:, gate_l, gate_s])
            nc.gpsimd.dma_start(out=natall[:, l], in_=emb[:, l])

        def stts(l, s0, s1):
            """fused square+row-sum for rows of chunks [s0, s1) of layer l.
            TensorScalarPtr is DVE-only on TRN2 (Pool rejects it)."""
            for s in range(s0, s1):
                junk = junkp.tile([128, D], bf16)
                nc.vector.scalar_tensor_tensor(
                    out=junk[:], in0=natall[:, l, s], scalar=1.0,
                    in1=natall[:, l, s], op0=OP.mult, op1=OP.mult,
                    accum_out=ssqa[:, l * SEG + s : l * SEG + s + 1])

        def lnexp(l, s0, s1):
            """inv_norm = exp(-0.5*ln(ssq)) for chunk columns [s0, s1)"""
            sl = slice(l * SEG + s0, l * SEG + s1)
            nc.scalar.activation(out=lnsa[:, sl], in_=ssqa[:, sl], func=AF.Ln)
            nc.scalar.activation(
                out=invna[:, sl], in_=lnsa[:, sl], func=AF.Exp, scale=-0.5)

        def muls(l, s0, s1):
            for s in range(s0, s1):
                c = l * SEG + s
                nc.vector.tensor_scalar_mul(
                    natnall[:, l, s], natall[:, l, s], invna[:, c : c + 1])

        def transpose(l, s0, s1):
            # xbar DMA transpose: xall[d, l, s, r] = natn[r, l, s, d]
            nc.sync.dma_start_transpose(
                out=xall[:, l, s0:s1], in_=natnall[:, l, s0:s1])

        def prep(l, slices):
            for a, b in slices:
                stts(l, a, b)
                lnexp(l, a, b)
                muls(l, a, b)
                transpose(l, a, b)

        def simtile_piece(l, rb, t, piece, ncols):
            """one PSUM tile: ncols columns starting at t*2048 + piece*1024.
            accum piece index q = t*2 + piece (dacc pre-zeroed)."""
            lr = l * 4 + rb
            stat_ap = xall[:, l, rb]              # [128(d), 128] local rows
            sim = simp.tile([128, 2048], f32, name="sim")
            for u in range(ncols // 512):
                cs = t * 4 + piece * 2 + u
                nc.tensor.matmul(
                    sim[:, u * 512 : (u + 1) * 512], stat_ap,
                    xall[:, l, cs * 4 : (cs + 1) * 4],
                    start=True, stop=True)
            eo = expp.tile([128, 2048], bf16, name="eo")
            q = t * 2 + piece
            nc.scalar.activation(
                out=eo[:, :ncols], in_=sim[:, :ncols], func=AF.Exp,
                scale=INV_T, accum_out=dacc[:, lr, q : q + 1])
            if piece == 0:
                eos[lr * 2 + t] = eo

        def simtiles(l, t, split_first=False):
            """one t-half of the 512x4096 sim row-block: bf16 matmuls +
            exp/rowsum on ACT. t=0 tiles only need the first transpose
            half, so all four run before any t=1 tile."""
            for rb in range(4):
                if rb == 0 and split_first:
                    # two 1024-wide pieces: the first only needs the s0-7
                    # quarter transposed, shortening the lead-in chain
                    simtile_piece(l, rb, t, 0, 1024)
                    simtile_piece(l, rb, t, 1, 1024)
                else:
                    simtile_piece(l, rb, t, 0, 2048)

        def extracts(l):
            """self (t=0) / positive (t=1) diagonals from exp-domain tiles."""
            for rb in range(4):
                lr = l * 4 + rb
                for t in range(2):
                    eo = eos[lr * 2 + t]
                    buf = sexpb if t == 0 else pexpb
                    junk = junkp.tile([128, 128], bf16)
                    nc.vector.scalar_tensor_tensor(
                        out=junk[:], in0=eo[:, rb * 128 : rb * 128 + 128],
                        scalar=1.0, in1=ident[:],
                        op0=OP.mult, op1=OP.mult,
                        accum_out=buf[:, lr : lr + 1])

        eos = {}
        # l=0 prep sliced fine to shorten the serial lead-in. prep(l+1) is
        # emitted after the t=0 tile group of simtiles(l): its small ACT
        # ops then sit behind 4 big exps (not in front of them), while its
        # DVE/Pool/DMA work schedules under l's exp phase. extracts(l) are
        # emitted one step late so they don't block the next layer's prep
        # in the in-order DVE queue.
        # tail tiles (shared by the two tail pieces)
        denom = singles.tile([128, 4 * L], f32)
        lnd = singles.tile([128, 4 * L], f32)
        lnp = singles.tile([128, 4 * L], f32)
        lp = singles.tile([128, 4 * L], f32)
        wlp = singles.tile([128, 4 * L], f32)

        def tail(l0_, l1_):
            """per-row loss terms for layers [l0_, l1_) + output DMA."""
            cs = slice(l0_ * 4, l1_ * 4)
            nc.vector.tensor_reduce(
                out=denom[:, cs], in_=dacc[:, cs, :], axis=AX.X, op=OP.add)
            nc.vector.tensor_sub(denom[:, cs], denom[:, cs], sexpb[:, cs])
            nc.scalar.activation(out=lnd[:, cs], in_=denom[:, cs], func=AF.Ln)
            nc.scalar.activation(out=lnp[:, cs], in_=pexpb[:, cs], func=AF.Ln)
            nc.vector.tensor_sub(lp[:, cs], lnd[:, cs], lnp[:, cs])
            for l in range(l0_, l1_):
                nc.vector.tensor_mul(
                    wlp[:, l * 4 : (l + 1) * 4],
                    lp[:, l * 4 : (l + 1) * 4], w[:])
            nc.sync.dma_start(out=out[:, cs], in_=wlp[:, cs])

        prep(0, [(0, 8), (8, 16), (16, 32)])
        natdma(1, 0, 15)
        for l in range(L):
            if l + 1 < L:
                prep(l + 1, [(0, 16), (16, 32)])
            simtiles(l, 0, split_first=(l == 0))
            if l == 0:
                natdma(2, 0, 31)
                natdma(3, 1, 15)
            if l >= 1:
                extracts(l - 1)
            if l == 2:
                # joint_valid weights, needed only by the tail; gated late
                # so the strided (slow) descriptor never sits ahead of
                # critical DMAs
                nc.gpsimd.tensor_copy(w[:], invna[:, 2 * SEG : 2 * SEG + 4])
                nc.gpsimd.dma_start(
                    out=w[:], in_=jvl.rearrange("(rb p) -> p rb", p=128))
            if l == 3:
                # loss columns for l=0..2 compute + stream out during l=3
                tail(0, 3)
            simtiles(l, 1)
        extracts(3)
        tail(3, 4)

    nc.finalize()
    _built = nc
    return nc


def _in_maps(emb_i, emb_j, joint_valid):
    import ml_dtypes

    emb_i = np.asarray(emb_i, dtype=np.float32)
    emb_j = np.asarray(emb_j, dtype=np.float32)
    jv = np.asarray(joint_valid, dtype=np.float32).reshape(-1)
    reps = np.concatenate(
        [emb_i.reshape(L, N, D), emb_j.reshape(L, N, D)], axis=1)  # [L, M, D]
    maps = []
    for c in range(NCORES):
        idx = (np.arange(M) + c * R) % M
        cols = reps[:, idx, :]  # rolled so local rows sit at columns 0..R-1
        nat = np.ascontiguousarray(
            cols.reshape(L, SEG, 128, D).transpose(2, 0, 1, 3)
        ).astype(ml_dtypes.bfloat16)
        jvl = np.ascontiguousarray(jv[(np.arange(R) + c * R) % N])
        maps.append({"emb_nat": nat, "jv_local": jvl})
    return maps, jv


def _combine(results, jv):
    tot = 0.0
    for r in results:
        tot += float(r["out_wlp"].astype(np.float64).sum())
    return np.float32(tot / (2.0 * float(jv.sum())))


def kernel(emb_i, emb_j, joint_valid):
    from concourse.bass_utils import run_bass_kernel_spmd

    nc = _build()
    maps, jv = _in_maps(emb_i, emb_j, joint_valid)
    res = run_bass_kernel_spmd(nc, maps, core_ids=list(range(NCORES)))
    return _combine(res.results, jv)


def run_traced(inputs, trace_cores=None):
    """test.py helper: same run but with NTFF tracing enabled."""
    from concourse.bass_utils import run_bass_kernel_spmd

    nc = _build()
    maps, jv = _in_maps(**inputs)
    res = run_bass_kernel_spmd(
        nc, maps, core_ids=list(range(NCORES)), trace=True,
        trace_cores=trace_cores if trace_cores is not None else list(range(NCORES)))
    res.loss = _combine(res.results, jv)
    return res


# revision 4
# speedup vs baseline: 1.0026x; 1.0026x over previous
"""Distributed Trainium2 (Bass/Tile) kernel for the contrastive loss, v2.

Strategy (8 NeuronCores, SPMD, row-sharded similarity matrix):
  Core c owns 512 of the 4096 rows of sim = reps @ reps^T (per l).
  The host rolls the column order by c*512 for each core so a single
  NEFF serves all cores, and pre-casts the embeddings to bf16.
  Each core, per l:
    - DMAs all 4096 raw bf16 embedding rows in [row, d] layout,
    - sums-of-squares per row (fused square+row-sum stt, DVE/Pool split),
    - inv-norm via exp(-0.5*ln(ssq)) on ACT (one table set),
    - normalizes rows (tensor_scalar per 128-row chunk, DVE/Pool split),
    - DMA-TRANSPOSES the normalized rows to [d, row] layout (xbar DMA
      transpose, no PE/PSUM involvement),
    - computes its 512x4096 sim row-block as bf16 matmuls (1 cyc/row)
      into two ping-pong [128, 2048] PSUM tiles (4 banks each),
    - exp(sim/T) on ACT with fused row-sum accum (denominator partials),
    - extracts the self/positive diagonals from the exp-domain SBUF
      output via masked stt against identity (DVE),
  then a small tail combines denom = sum - selfexp, ln's on ACT, and
  weights by joint_valid; host sums the 8 partial tensors (all-reduce).

  The prep work (ssq/invn/normalize/transpose) for layer l+1 is emitted
  one step ahead of simexp(l) so it runs on otherwise-idle DVE/Pool/DMA
  capacity underneath the ACT-bound exp phase; l=0's prep is sliced into
  quarters to shorten the serial lead-in chain.
"""

import numpy as np

TEMP = 0.2
L, B, K, D = 4, 64, 32, 128
N = B * K          # 2048
M = 2 * N          # 4096 rows of sim per l
NCORES = 8
R = M // NCORES    # 512 local rows per core
SEG = M // 128     # 32 row-tiles of 128 per l
INV_T = 1.0 / TEMP

_built = None


def _build():
    global _built
    if _built is not None:
        return _built
    from contextlib import ExitStack

    import concourse.tile as tile
    from concourse import bacc
    import concourse.mybir as mybir
    from concourse.masks import make_identity

    f32 = mybir.dt.float32
    bf16 = mybir.dt.bfloat16
    AF = mybir.ActivationFunctionType
    OP = mybir.AluOpType
    AX = mybir.AxisListType

    # Pin every ACT op to the natural_log_exp_and_others table set (covers
    # Copy/Exp/Identity/Ln/Square), so bacc emits exactly one LoadActFuncSet.
    from concourse import hw_specs as _hw
    _tabs = dict(_hw.get_activation_tables("gen3"))
    _pinned = {
        name: (fns if name == "natural_log_exp_and_others" else frozenset())
        for name, fns in _tabs.items()
    }
    _hw.get_activation_tables.cache_clear()
    _orig = _hw.get_activation_tables.__wrapped__

    def _patched(arch):
        if arch == "gen3":
            return _pinned
        return _orig(arch)

    _hw.get_activation_tables = _patched
    import concourse.bacc as _baccmod
    if hasattr(_baccmod, "get_activation_tables"):
        _baccmod.get_activation_tables = _patched

    nc = bacc.Bacc(None, target_bir_lowering=False)
    emb = nc.dram_tensor("emb_nat", [128, L, SEG, D], bf16, kind="ExternalInput")
    jvl = nc.dram_tensor("jv_local", [R], f32, kind="ExternalInput")
    out = nc.dram_tensor("out_wlp", [128, 4 * L], f32, kind="ExternalOutput")

    with ExitStack() as ctx:
        tc = ctx.enter_context(tile.TileContext(nc))
        singles = ctx.enter_context(tc.tile_pool(name="singles", bufs=1))
        junkp = ctx.enter_context(tc.tile_pool(name="junk", bufs=6))
        sqp = ctx.enter_context(tc.tile_pool(name="sqp", bufs=2))
        expp = ctx.enter_context(tc.tile_pool(name="expo", bufs=12))
        simp = ctx.enter_context(tc.tile_pool(name="sim", bufs=2, space="PSUM"))
        sqs = {}

        ident = singles.tile([128, 128], bf16)
        make_identity(nc, ident[:])

        # dummy ACT op on an immediately-ready tile: pulls the (single)
        # LoadActFuncSet to t~0 instead of mid-way into the l=0 prep chain
        actwarm = singles.tile([128, 128], f32)
        nc.scalar.activation(out=actwarm[:], in_=ident[:], func=AF.Exp)

        w = singles.tile([128, 4], f32)

        natall = singles.tile([128, L, SEG, D], bf16)    # raw rows
        natnall = singles.tile([128, L, SEG, D], bf16)   # normalized rows
        # normalized reps, transposed to [d, (l, s, row)] layout
        xall = singles.tile([128, L, SEG, 128], bf16)
        ssqa = singles.tile([128, L * SEG], f32)
        lnsa = singles.tile([128, L * SEG], f32)
        invna = singles.tile([128, L * SEG], f32)
        # exp row-sum partials: up to 4 tile-pieces per (l, rb)
        dacc = singles.tile([128, 4 * L, 4], f32)
        nc.gpsimd.memset(dacc[:], 0.0)
        sexpb = singles.tile([128, 4 * L], f32)     # exp(self/T)
        pexpb = singles.tile([128, 4 * L], f32)     # exp(pos/T)

        # l=0 arrives in quarter/half slices on the SP HWDGE queue so its
        # first ssq ops can start ~2us in. The later nat DMAs are emitted
        # interleaved with the l0/l1 transposes (SP queue order == issue
        # order) so they don't hog the shared DMA engines during lead-in.
        for a, b in ((0, 8), (8, 16), (16, 32)):
            nc.sync.dma_start(out=natall[:, 0, a:b], in_=emb[:, 0, a:b])

        def natdma(l, gate_l, gate_s):
            """Input DMA for layer l, gated so it cannot transfer before
            chunk (gate_l, gate_s) of the normalize stage is written. The
            dummy copy creates a WAW dependency into the DMA's own output
            region (the DMA then overwrites it), which is the only gating
            the greedy tile scheduler respects — queue order is not FIFO.
            Without this, these dependency-free DMAs hog the shared DMA
            engines ahead of the critical l0 transposes."""
            nc.gpsimd.tensor_copy(natall[:, l, 0], natnall[:, gate_l, gate_s])
            nc.gpsimd.dma_start(out=natall[:, l], in_=emb[:, l])

        def stts(l, s0, s1):
            """fused square+row-sum for rows of chunks [s0, s1) of layer l.
            TensorScalarPtr is DVE-only on TRN2 (Pool rejects it)."""
            for s in range(s0, s1):
                junk = junkp.tile([128, D], bf16)
                nc.vector.scalar_tensor_tensor(
                    out=junk[:], in0=natall[:, l, s], scalar=1.0,
                    in1=natall[:, l, s], op0=OP.mult, op1=OP.mult,
                    accum_out=ssqa[:, l * SEG + s : l * SEG + s + 1])

        def lnexp(l, s0, s1):
            """inv_norm = exp(-0.5*ln(ssq)) for chunk columns [s0, s1)"""
            sl = slice(l * SEG + s0, l * SEG + s1)
            nc.scalar.activation(out=lnsa[:, sl], in_=ssqa[:, sl], func=AF.Ln)
            nc.scalar.activation(
                out=invna[:, sl], in_=lnsa[:, sl], func=AF.Exp, scale=-0.5)

        def muls(l, s0, s1):
            for s in range(s0, s1):
                c = l * SEG + s
                nc.vector.tensor_scalar_mul(
                    natnall[:, l, s], natall[:, l, s], invna[:, c : c + 1])

        def transpose(l, s0, s1):
            # xbar DMA transpose: xall[d, l, s, r] = natn[r, l, s, d]
            nc.sync.dma_start_transpose(
                out=xall[:, l, s0:s1], in_=natnall[:, l, s0:s1])

        def prep(l, slices):
            for a, b in slices:
                stts(l, a, b)
                lnexp(l, a, b)
                muls(l, a, b)
                transpose(l, a, b)

        def simtile_piece(l, rb, t, piece, ncols):
            """one PSUM tile: ncols columns starting at t*2048 + piece*1024.
            accum piece index q = t*2 + piece (dacc pre-zeroed)."""
            lr = l * 4 + rb
            stat_ap = xall[:, l, rb]              # [128(d), 128] local rows
            sim = simp.tile([128, 2048], f32, name="sim")
            for u in range(ncols // 512):
                cs = t * 4 + piece * 2 + u
                nc.tensor.matmul(
                    sim[:, u * 512 : (u + 1) * 512], stat_ap,
                    xall[:, l, cs * 4 : (cs + 1) * 4],
                    start=True, stop=True)
            eo = expp.tile([128, 2048], bf16, name="eo")
            q = t * 2 + piece
            nc.scalar.activation(
                out=eo[:, :ncols], in_=sim[:, :ncols], func=AF.Exp,
                scale=INV_T, accum_out=dacc[:, lr, q : q + 1])
            if piece == 0:
                eos[lr * 2 + t] = eo

        def simtiles(l, t, split_first=False):
            """one t-half of the 512x4096 sim row-block: bf16 matmuls +
            exp/rowsum on ACT. t=0 tiles only need the first transpose
            half, so all four run before any t=1 tile."""
            for rb in range(4):
                if rb == 0 and split_first:
                    # two 1024-wide pieces: the first only needs the s0-7
                    # quarter transposed, shortening the lead-in chain
                    simtile_piece(l, rb, t, 0, 1024)
                    simtile_piece(l, rb, t, 1, 1024)
                else:
                    simtile_piece(l, rb, t, 0, 2048)

        def extracts(l):
            """self (t=0) / positive (t=1) diagonals from exp-domain tiles."""
            for rb in range(4):
                lr = l * 4 + rb
                for t in range(2):
                    eo = eos[lr * 2 + t]
                    buf = sexpb if t == 0 else pexpb
                    junk = junkp.tile([128, 128], bf16)
                    nc.vector.scalar_tensor_tensor(
                        out=junk[:], in0=eo[:, rb * 128 : rb * 128 + 128],
                        scalar=1.0, in1=ident[:],
                        op0=OP.mult, op1=OP.mult,
                        accum_out=buf[:, lr : lr + 1])

        eos = {}
        # l=0 prep sliced fine to shorten the serial lead-in. prep(l+1) is
        # emitted after the t=0 tile group of simtiles(l): its small ACT
        # ops then sit behind 4 big exps (not in front of them), while its
        # DVE/Pool/DMA work schedules under l's exp phase. extracts(l) are
        # emitted one step late so they don't block the next layer's prep
        # in the in-order DVE queue.
        # tail tiles (shared by the two tail pieces)
        denom = singles.tile([128, 4 * L], f32)
        lnd = singles.tile([128, 4 * L], f32)
        lnp = singles.tile([128, 4 * L], f32)
        lp = singles.tile([128, 4 * L], f32)
        wlp = singles.tile([128, 4 * L], f32)

        def tail(l0_, l1_):
            """per-row loss terms for layers [l0_, l1_) + output DMA."""
            cs = slice(l0_ * 4, l1_ * 4)
            nc.vector.tensor_reduce(
                out=denom[:, cs], in_=dacc[:, cs, :], axis=AX.X, op=OP.add)
            nc.vector.tensor_sub(denom[:, cs], denom[:, cs], sexpb[:, cs])
            nc.scalar.activation(out=lnd[:, cs], in_=denom[:, cs], func=AF.Ln)
            nc.scalar.activation(out=lnp[:, cs], in_=pexpb[:, cs], func=AF.Ln)
            nc.vector.tensor_sub(lp[:, cs], lnd[:, cs], lnp[:, cs])
            for l in range(l0_, l1_):
                nc.vector.tensor_mul(
                    wlp[:, l * 4 : (l + 1) * 4],
                    lp[:, l * 4 : (l + 1) * 4], w[:])
            nc.sync.dma_start(out=out[:, cs], in_=wlp[:, cs])

        prep(0, [(0, 8), (8, 16), (16, 32)])
        natdma(1, 0, 15)
        for l in range(L):
            if l + 1 < L:
                prep(l + 1, [(0, 16), (16, 32)])
            simtiles(l, 0, split_first=(l <= 1))
            if l == 0:
                natdma(2, 0, 31)
                natdma(3, 1, 15)
            if l >= 1:
                extracts(l - 1)
            if l == 2:
                # joint_valid weights, needed only by the tail; gated late
                # so the strided (slow) descriptor never sits ahead of
                # critical DMAs
                nc.gpsimd.tensor_copy(w[:], invna[:, 2 * SEG : 2 * SEG + 4])
                nc.gpsimd.dma_start(
                    out=w[:], in_=jvl.rearrange("(rb p) -> p rb", p=128))
            if l == 3:
                # loss columns for l=0..2 compute + stream out during l=3
                tail(0, 3)
            simtiles(l, 1)
        extracts(3)
        tail(3, 4)

    nc.finalize()
    _built = nc
    return nc


def _in_maps(emb_i, emb_j, joint_valid):
    import ml_dtypes

    emb_i = np.asarray(emb_i, dtype=np.float32)
    emb_j = np.asarray(emb_j, dtype=np.float32)
    jv = np.asarray(joint_valid, dtype=np.float32).reshape(-1)
    reps = np.concatenate(
        [emb_i.reshape(L, N, D), emb_j.reshape(L, N, D)], axis=1)  # [L, M, D]
    maps = []
    for c in range(NCORES):
        idx = (np.arange(M) + c * R) % M
        cols = reps[:, idx, :]  # rolled so local rows sit at columns 0..R-1
        nat = np.ascontiguousarray(
            cols.reshape(L, SEG, 128, D).transpose(2, 0, 1, 3)
        ).astype(ml_dtypes.bfloat16)
        jvl = np.ascontiguousarray(jv[(np.arange(R) + c * R) % N])
        maps.append({"emb_nat": nat, "jv_local": jvl})
    return maps, jv


def _combine(results, jv):
    tot = 0.0
    for r in results:
        tot += float(r["out_wlp"].astype(np.float64).sum())
    return np.float32(tot / (2.0 * float(jv.sum())))


def kernel(emb_i, emb_j, joint_valid):
    from concourse.bass_utils import run_bass_kernel_spmd

    nc = _build()
    maps, jv = _in_maps(emb_i, emb_j, joint_valid)
    res = run_bass_kernel_spmd(nc, maps, core_ids=list(range(NCORES)))
    return _combine(res.results, jv)


def run_traced(inputs, trace_cores=None):
    """test.py helper: same run but with NTFF tracing enabled."""
    from concourse.bass_utils import run_bass_kernel_spmd

    nc = _build()
    maps, jv = _in_maps(**inputs)
    res = run_bass_kernel_spmd(
        nc, maps, core_ids=list(range(NCORES)), trace=True,
        trace_cores=trace_cores if trace_cores is not None else list(range(NCORES)))
    res.loss = _combine(res.results, jv)
    return res
